# revision 22
# baseline (speedup 1.0000x reference)
"""Additive (Bahdanau) attention on 8 TRN2 NeuronCores, data-parallel.

Full problem: queries (4,256,256), keys (4,1024,256), values (4,256,1024),
W_q (256,128), W_k (256,128), w_v (128,) ->
    out[b,q,d] = softmax_k( sum_h w_v[h]*tanh((q W_q)[b,q,h]+(k W_k)[b,k,h]) ) @ values[b,d,:]^T

Sharding: 8 cores = (batch b, half of Q). Each core handles 128 queries with
its batch's full keys/values. No collectives needed.

Algorithm: the tanh feature tensor (Q*K*H elements) is never materialized.
tanh(s) ~= alpha*s + sum_r a_r sin(r*w*s)  (harmonics r=1..4 of one
fundamental): sin(w(x+y)) expands into separable sin/cos products, so scores
become PE matmuls with contraction H per (harmonic, trig) pair, plus a linear
term computed straight from keys with a host-precomputed broadcast stationary
u = W_k @ w_v.  Only the FUNDAMENTAL needs feature matmuls and ACT sins;
harmonics 2..4 are derived on DVE with double/triple/quadruple-angle algebra
in raw form (s1*c1, s1^2, (s1^2-.75)s1, ...), with the angle-formula
constants folded into the per-(r,t) Q-side fold scales and softmax-invariant
per-q constants dropped.

Range reduction for the ACT Sin spline ([-pi,pi] only), in "turns" units:
DVE computes n = round(v) via the fp32 magic constant, f = v - n, and the
cosine argument z = f + (f < 0.25) in one fused scalar_tensor_tensor,
consumed as sin(2*pi*z - 3*pi/2) = cos(2*pi*v)  (z in [0.25, 1.25)).
"""

import os
import sys
import types

import numpy as np

# ---------------------------------------------------------------------------
# antenv.axon_hooks shim: the image's antenv package lacks axon_hooks, which
# run_bass_kernel_spmd(trace=True) imports for NTFF profiling under axon.
if "antenv.axon_hooks" not in sys.modules:
    _m = types.ModuleType("antenv.axon_hooks")
    _m._hook = None
    _m.set_axon_ntff_profile_hook = lambda h: setattr(_m, "_hook", h)
    _m.get_axon_ntff_profile_hook = lambda: _m._hook
    sys.modules["antenv.axon_hooks"] = _m
    try:
        from trn_agent_boot.trn_boot import _ntff_profile_via_ctypes

        _m.set_axon_ntff_profile_hook(
            _ntff_profile_via_ctypes("/opt/axon/libaxon_pjrt.so")
        )
    except Exception:
        pass

import concourse.bass as bass
import concourse.tile as tile
from concourse import mybir
from concourse.bass_utils import run_bass_kernel_spmd
from concourse.vector_clock import ScopedClock

# ---------------------------------------------------------------------------
# This walrus build rejects >1 sync-wait command on one instruction; Tile's
# kernel-tail drain accumulates one wait per outstanding semaphore. Split the
# overflow onto follow-up SP nops.
_MAX_WAITS = 1


def _patched_drain_and_barrier(self, tick_clock, wait_clock):
    """Minimal kernel tail: drain with its sem waits only.  The NEFF is
    load->execute-once->unload per call, so the end-of-kernel barrier and
    semaphore cleanup (7+us of EVENT_SEMAPHORE ping-pong) are dead weight."""
    nc = self.nc
    drain_inst = nc.sync.drain()
    wait_clock.add_sem_waits(
        drain_inst.ins, ScopedClock({None: tick_clock.global_clock})
    )
    si = drain_inst.ins.sync_info
    if si is not None and len(si.on_wait) > _MAX_WAITS:
        waits = list(si.on_wait)
        drain_inst.ins.sync_info = mybir.SyncInfo(
            on_wait=waits[:_MAX_WAITS], on_update=list(si.on_update)
        )
        for k in range(_MAX_WAITS, len(waits), _MAX_WAITS):
            extra = nc.sync.nop()
            extra.ins.sync_info = mybir.SyncInfo(
                on_wait=waits[k : k + _MAX_WAITS], on_update=[]
            )
    assert self.sems is not None
    popped = nc._tile_sem_poison_stack.pop()
    assert popped is self._sem_poison


tile.TileContext._drain_and_barrier = _patched_drain_and_barrier

_nopctr = 0


def _split_multi_waits(nc, max_waits=_MAX_WAITS):
    """Walrus here allows only one sem-wait command per instruction; move
    extra waits onto preceding same-engine NOPs (semantically identical:
    the engine blocks on each wait in order)."""
    global _nopctr
    for f in nc.m.functions:
        for bb in f.blocks:
            insts = bb.instructions
            out = []
            changed = False
            for inst in insts:
                si = inst.sync_info
                if si is not None and len(si.on_wait) > max_waits:
                    changed = True
                    waits = list(si.on_wait)
                    n_extra = len(waits) - max_waits
                    for k in range(0, n_extra, max_waits):
                        nop = mybir.InstNoOp(name=f"waitsplit_{_nopctr}", ins=[], outs=[])
                        _nopctr += 1
                        nop.engine = inst.engine
                        nop.sync_info = mybir.SyncInfo(
                            on_wait=waits[k : min(k + max_waits, n_extra)], on_update=[]
                        )
                        out.append(nop)
                    inst.sync_info = mybir.SyncInfo(
                        on_wait=waits[n_extra:], on_update=list(si.on_update)
                    )
                out.append(inst)
            if changed:
                bb.instructions = out


# ---------------------------------------------------------------------------
B, Q, K = 4, 256, 1024
I, H, D = 256, 128, 256  # input dim, hidden dim, value dim
QL = 128  # queries per core
KH = 512  # K half
KC = K // 128  # 8 key chunks of 128
N_CORES = 8
R = 3  # number of harmonics
F32 = mybir.dt.float32
F16 = mybir.dt.float16
F8 = mybir.dt.float8e4

# Weighted LS fit of tanh(s) ~= ALPHA*s + sum_r COEFS[r]*sin((r+1)*W0*s)
# over s ~ N(0, 1.55); end-to-end rel err ~5e-3 (f16).
W0 = 0.8911
COEFS = np.array([0.4814, 0.1000, 0.0351], dtype=np.float64)
ALPHA = 0.2782
C1R0 = float(np.float32(W0 / (2 * np.pi)))  # "turns" scale, folded into W

SCORE_FP8 = False  # fp8e4 DoubleRow score matmuls (2x PE) at ~1e-2 rel err
FQS = 16.0 if SCORE_FP8 else 1.0  # Fq prescale (fp8 subnormal avoidance)
N_WARM = 12   # PE p-state warm-up matmuls before real work
N_FILL_B = 10  # fillers between vq and vk (cover keysT wait)
N_FILL_C = 8   # fillers before the score block (keep clock high)
N_FILL_D = 8   # fillers between the h0 and h1 score blocks
N_FILL_E = 8   # fillers during the exp phase (keep clock for out matmuls)

TPI = float(2 * np.pi)
PI = float(np.pi)
KMAG = float(np.float32(1.5 * 2**23))  # fp32 round-to-nearest magic constant
CAW = 416  # constA c-tile: 128 wq0 | 128 q | 128 wk0 | 32 f32-const slots

_nc_cache = None


def build():
    nc = bass.Bass("TRN2", target_bir_lowering=False, debug=False, num_devices=N_CORES)
    constA_ext = nc.declare_dram_parameter("constA", [128, 2, CAW], F16, isOutput=False)
    constB_ext = nc.declare_dram_parameter("constB", [128, 384], F16, isOutput=False)
    keysT_ext = nc.declare_dram_parameter("keysT", [128, 2, 2, KH], F16, isOutput=False)
    valT_ext = nc.declare_dram_parameter("valT", [128, KC, D], F16, isOutput=False)
    out_ext = nc.declare_dram_parameter("out", [QL, D], F16, isOutput=True)

    with tile.TileContext(nc) as tc:
        _build_body(nc, tc, constA_ext, constB_ext, keysT_ext, valT_ext, out_ext)
    _split_multi_waits(nc)
    return nc


def _build_body(nc, tc, constA_ext, constB_ext, keysT_ext, valT_ext, out_ext):
    A = mybir.AluOpType
    AF = mybir.ActivationFunctionType
    FD8 = F8 if SCORE_FP8 else F16
    with (
        tc.tile_pool(name="const", bufs=1) as constp,
        tc.tile_pool(name="big", bufs=1) as bigp,
        tc.tile_pool(name="wrk", bufs=2) as wrkp,
        tc.tile_pool(name="scoresp", bufs=1, space="PSUM") as scoresp,
        tc.tile_pool(name="vkp", bufs=2, space="PSUM") as vkp,
        tc.tile_pool(name="vqp", bufs=1, space="PSUM") as vqp,
        tc.tile_pool(name="tpp", bufs=1, space="PSUM") as tpp,
        tc.tile_pool(name="outp", bufs=1, space="PSUM") as outp,
    ):
        # ---- SBUF tiles --------------------------------------------------
        constA = constp.tile([128, 2, CAW], F16)  # wq0 | queriesT | wk0 | consts
        constB = constp.tile([128, 384], F16)     # ulin c0 | ulin c1 | ident
        keysT = bigp.tile([128, 2, 2, KH], F16)   # [p, khalf, c, k']
        valT = bigp.tile([128, KC, D], F16)
        scratch_sb = bigp.tile([128, 256], F16)
        argsq = bigp.tile([128, 2, QL], F16)      # plane1=f, plane0=z
        argsk = bigp.tile([128, 2, K], F16)       # plane1=f, plane0=z
        # Fqraw[r,t] = raw q-side factor for pair (r,t); Fq = scale*raw (+bias)
        Fqraw = bigp.tile([128, R, 2, QL], F16)
        Fq = bigp.tile([128, R, 2, QL], FD8)
        # G[r,t] = raw k-side partner of pair (r,t):
        #   r0: (c1k, s1k); r1: (Uk, Pk); r2: (p3ck, p3k)
        G = bigp.tile([128, R, 2, K], FD8)
        u2q = bigp.tile([128, QL], F16)
        attn = bigp.tile([128, K], F16)
        attnT = bigp.tile([128, KC, QL], F16)
        esum = bigp.tile([128, 2], F32)
        sums = bigp.tile([128, 1], F32)
        rs = bigp.tile([128, 1], F32)
        out_sb = bigp.tile([128, D], F16)
        atl_dummy = bigp.tile([128, 1], F16)

        # ---- PSUM tiles --------------------------------------------------
        scores_ps = scoresp.tile([128, K], F32)       # 2 banks
        vq_ps = vqp.tile([128, QL], F32)
        tpall = tpp.tile([128, K], F16)               # 8 transposes, 1 bank
        out_ps = outp.tile([128, D], F32)             # also the filler target

        wq0 = lambda c: constA[:, c, 0:128]
        queriesT = lambda c: constA[:, c, 128:256]
        wk0 = lambda c: constA[:, c, 256:384]
        ulin = lambda c: constB[:, c * 128 : (c + 1) * 128]
        ident = constB[:, 256:384]
        fc = lambda i: constA[:, 0, 384 + 2 * i : 386 + 2 * i].bitcast(F32)
        bnpi = fc(0)                    # -pi
        fsc = lambda r, t: fc(1 + 2 * r + t)   # fold scales (r,t)
        fbias1 = fc(7)                  # fold bias for (1,1)
        bnhpi = fc(8)                   # -pi/2

        # ---- input DMAs: triggers spread across engines so descriptor
        # generation (~0.65us each) runs in parallel --------------------
        nc.sync.dma_start(out=constA[:], in_=constA_ext[:])
        nc.scalar.dma_start(out=keysT[:, 0:1], in_=keysT_ext[:, 0:1])
        nc.gpsimd.dma_start(out=keysT[:, 1:2], in_=keysT_ext[:, 1:2])
        nc.sync.dma_start(out=constB[:], in_=constB_ext[:])

        # ---- PE: p-state warm-up fillers during the DMA window -----------
        nc.gpsimd.memset(scratch_sb[:], 0)

        def fillers(n):
            for _ in range(n):
                nc.tensor.matmul(
                    out_ps[:, 0:128], scratch_sb[:, 0:128],
                    scratch_sb[:, 0:128], start=True, stop=True,
                )

        fillers(N_WARM)

        # ---- PE: vq, vk, ulin -------------------------------------------
        for c in range(2):
            nc.tensor.matmul(
                vq_ps[:], wq0(c), queriesT(c), start=(c == 0), stop=(c == 1)
            )
        fillers(N_FILL_B)
        vk_tiles = {}
        vk_insts = {}
        for h in range(2):
            vk_t = vkp.tile([128, KH], F32, tag="vk", name=f"vk_{h}")
            vk_tiles[h] = vk_t
            for c in range(2):
                vk_insts[(h, c)] = nc.tensor.matmul(
                    vk_t[:], wk0(c), keysT[:, h, c, :],
                    start=(c == 0), stop=(c == 1),
                )
        # linear term straight from keys: scores[q,k] += FQS*alpha*u.keys
        for h in range(2):
            lo, hi = h * KH, (h + 1) * KH
            for c in range(2):
                nc.tensor.matmul(
                    scores_ps[:, lo:hi], ulin(c), keysT[:, h, c, :],
                    start=(c == 0), stop=False,
                )
        fillers(N_FILL_C)

        # ---- DVE: single-wrap args; |v| <= 0.70 < 1 so one conditional
        # +1 suffices: ys = (v<0)+v in [0,1), sin(2pi*ys - pi) = -sin(2pi*v);
        # yc = (v<-0.25)+v in [-0.25,0.75], sin(2pi*yc - pi/2) = -cos(2pi*v).
        # Every score pair has an even number of sign-flipped factors, so the
        # minus signs cancel identically.
        vqs = wrkp.tile([128, QL], F16, tag="vqs")
        nc.vector.tensor_copy(vqs[:], vq_ps[:])
        nc.vector.scalar_tensor_tensor(
            argsq[:, 1, :], vqs[:], 0.0, vqs[:], A.is_lt, A.add
        )
        nc.vector.scalar_tensor_tensor(
            argsq[:, 0, :], vqs[:], -0.25, vqs[:], A.is_lt, A.add
        )
        for h in range(2):
            lo, hi = h * KH, (h + 1) * KH
            vks = wrkp.tile([128, KH], F16, tag="vks", name=f"vks_{h}")
            nc.vector.tensor_copy(vks[:], vk_tiles[h][:])
            nc.vector.scalar_tensor_tensor(
                argsk[:, 1, lo:hi], vks[:], 0.0, vks[:], A.is_lt, A.add
            )
            nc.vector.scalar_tensor_tensor(
                argsk[:, 0, lo:hi], vks[:], -0.25, vks[:], A.is_lt, A.add
            )

        # values DMA: deferred so it doesn't steal HBM bandwidth at startup
        vdma = nc.gpsimd.dma_start(out=valT[:], in_=valT_ext[:])
        tile.add_dep_helper(
            vdma.ins, vk_insts[(1, 1)].ins,
            sync=True, reason="defer values DMA past startup",
        )

        # ---- ACT q sins, then DVE q derived raws -------------------------
        s1q, c1q = Fqraw[:, 0, 0, :], Fqraw[:, 0, 1, :]
        Pq, Uq = Fqraw[:, 1, 0, :], Fqraw[:, 1, 1, :]
        p3q, p3cq = Fqraw[:, 2, 0, :], Fqraw[:, 2, 1, :]
        nc.scalar.activation(s1q, argsq[:, 1, :], AF.Sin, scale=TPI, bias=bnpi)
        nc.scalar.activation(c1q, argsq[:, 0, :], AF.Sin, scale=TPI, bias=bnhpi)
        nc.vector.tensor_tensor(Pq, s1q, c1q, A.mult)
        nc.vector.tensor_tensor(Uq, s1q, s1q, A.mult)
        nc.vector.scalar_tensor_tensor(p3q, Uq, -0.75, s1q, A.add, A.mult)
        nc.vector.tensor_tensor(u2q[:], c1q, c1q, A.mult)
        nc.vector.scalar_tensor_tensor(p3cq, u2q[:], -0.75, c1q, A.add, A.mult)
        # DVE folds (pure scales); the affine (1,1) fold runs on ACT below
        for r, t in [(0, 0), (0, 1), (1, 0), (2, 0), (2, 1)]:
            nc.vector.tensor_scalar_mul(
                Fq[:, r, t, :], Fqraw[:, r, t, :], fsc(r, t)
            )

        # ---- per K-half: ACT sins+squares, DVE products ------------------
        for h in range(2):
            lo, hi = h * KH, (h + 1) * KH
            c1k, s1k = G[:, 0, 0, lo:hi], G[:, 0, 1, lo:hi]
            Uk, Pk = G[:, 1, 0, lo:hi], G[:, 1, 1, lo:hi]
            p3ck, p3k = G[:, 2, 0, lo:hi], G[:, 2, 1, lo:hi]
            nc.scalar.activation(s1k, argsk[:, 1, lo:hi], AF.Sin, scale=TPI, bias=bnpi)
            nc.scalar.activation(
                c1k, argsk[:, 0, lo:hi], AF.Sin, scale=TPI, bias=bnhpi
            )
            if h == 1:
                # dummy exp right after the LAST Sin: hoists the exp-table
                # load here; Square/Identity exist in both table sets.
                nc.scalar.activation(atl_dummy[:], scratch_sb[:, 0:1], AF.Exp)
            nc.scalar.activation(Uk, s1k, AF.Square)
            u2k = wrkp.tile([128, KH], F16, tag="u2k", name=f"u2k_{h}")
            nc.vector.tensor_tensor(u2k[:], c1k, c1k, A.mult)
            if h == 0:
                nc.scalar.activation(
                    Fq[:, 1, 1, :], Fqraw[:, 1, 1, :], AF.Identity,
                    scale=fsc(1, 1), bias=fbias1,
                )
            nc.vector.tensor_tensor(Pk, s1k, c1k, A.mult)
            nc.vector.scalar_tensor_tensor(p3k, Uk, -0.75, s1k, A.add, A.mult)
            nc.vector.scalar_tensor_tensor(p3ck, u2k[:], -0.75, c1k, A.add, A.mult)

        # ---- PE score matmuls: h0 block strictly before h1 block ---------
        smm = []
        for h in range(2):
            lo, hi = h * KH, (h + 1) * KH
            if h == 1:
                fillers(N_FILL_D)
            if SCORE_FP8:
                for r in range(R):
                    smm.append(nc.tensor.matmul(
                        scores_ps[:, lo:hi], Fq[:, r, :, :], G[:, r, :, lo:hi],
                        start=False, stop=(r == R - 1),
                        perf_mode=mybir.MatmulPerfMode.DoubleRow,
                    ))
            else:
                for r in range(R):
                    for t in range(2):
                        smm.append(nc.tensor.matmul(
                            scores_ps[:, lo:hi], Fq[:, r, t, :], G[:, r, t, lo:hi],
                            start=False, stop=(r == R - 1 and t == 1),
                        ))
        n_blk = len(smm) // 2
        # order-only dep: h1 matmuls must not be scheduled before h0's stop,
        # else the h0 exp is gated on the whole score phase
        tile.add_dep_helper(
            smm[n_blk].ins, smm[n_blk - 1].ins, sync=False,
            reason="keep h0 scores closing before h1 block",
        )

        # ---- softmax + attn^T + out, pipelined per K-half ----------------
        fillers(N_FILL_E)
        for h in range(2):
            lo, hi = h * KH, (h + 1) * KH
            nc.scalar.activation(
                attn[:, lo:hi], scores_ps[:, lo:hi], AF.Exp, scale=1.0 / FQS,
                accum_out=esum[:, h : h + 1],
            )
            for kc in range(4 * h, 4 * h + 4):
                nc.tensor.transpose(
                    tpall[:, kc * 128 : (kc + 1) * 128],
                    attn[:, kc * 128 : (kc + 1) * 128],
                    ident,
                )
            nc.vector.tensor_copy(attnT[:, 4 * h : 4 * h + 4, :], tpall[:, lo:hi])
            if h == 0:
                for kc in range(0, 4):
                    for dg in range(2):
                        # start=True zeroes the whole 2KB bank: only the very
                        # first matmul into out_ps may carry it
                        nc.tensor.matmul(
                            out_ps[:, dg * 128 : (dg + 1) * 128],
                            attnT[:, kc, :], valT[:, kc, dg * 128 : (dg + 1) * 128],
                            start=(kc == 0 and dg == 0), stop=False,
                        )
        # h1: finish the d0 accumulation first so its scale+DMA (descriptor
        # generation ~0.6us) overlaps the d1 matmuls
        nc.vector.tensor_tensor(sums[:], esum[:, 0:1], esum[:, 1:2], A.add)
        nc.vector.reciprocal(rs[:], sums[:])
        for kc in range(4, 8):
            nc.tensor.matmul(
                out_ps[:, 0:128], attnT[:, kc, :], valT[:, kc, 0:128],
                start=False, stop=(kc == 7),
            )
        nc.vector.tensor_scalar_mul(out_sb[:, 0:128], out_ps[:, 0:128], rs[:])
        nc.sync.dma_start(out=out_ext[:, 0:128], in_=out_sb[:, 0:128])
        for kc in range(4, 8):
            nc.tensor.matmul(
                out_ps[:, 128:256], attnT[:, kc, :], valT[:, kc, 128:256],
                start=False, stop=(kc == 7),
            )
        nc.vector.tensor_scalar_mul(out_sb[:, 128:256], out_ps[:, 128:256], rs[:])
        dbg = os.environ.get("KDBG", "")
        if dbg == "attn":
            nc.sync.dma_start(out=out_ext[:], in_=attn[:, 0:256])
        elif dbg == "argf":
            nc.sync.dma_start(out=out_ext[:], in_=argsk[:, 1, 0:256])
        elif dbg == "argz":
            nc.sync.dma_start(out=out_ext[:], in_=argsk[:, 0, 0:256])
        elif dbg == "s1k":
            nc.sync.dma_start(out=out_ext[:], in_=G[:, 0, 1, 0:256])
        elif dbg == "c1k":
            nc.sync.dma_start(out=out_ext[:], in_=G[:, 0, 0, 0:256])
        elif dbg == "pk":
            nc.sync.dma_start(out=out_ext[:], in_=G[:, 1, 1, 0:256])
        elif dbg == "fq":
            nc.sync.dma_start(out=out_ext[:], in_=Fq[:, 0, :, :])
        elif dbg == "fqraw":
            nc.sync.dma_start(out=out_ext[:], in_=Fqraw[:, 0, :, :])
        elif dbg == "uk":
            nc.sync.dma_start(out=out_ext[:], in_=G[:, 1, 0, 0:256])
        else:
            nc.sync.dma_start(out=out_ext[:, 128:256], in_=out_sb[:, 128:256])


def _make_in_maps(queries, keys, values, W_q, W_k, w_v):
    queries = np.asarray(queries, np.float32)
    keys = np.asarray(keys, np.float32)
    values = np.asarray(values, np.float32)
    W_q = np.asarray(W_q, np.float32)
    W_k = np.asarray(W_k, np.float32)
    w_v = np.asarray(w_v, np.float32).reshape(H)

    # fundamental-only stationaries in "turns": wq0[p,c,h] = W_q[c*128+p,h]*C1R0
    wq0 = np.ascontiguousarray(
        (W_q * C1R0).reshape(2, 128, H).transpose(1, 0, 2)
    ).astype(np.float16)
    wk0 = np.ascontiguousarray(
        (W_k * C1R0).reshape(2, 128, H).transpose(1, 0, 2)
    ).astype(np.float16)

    u = (W_k.astype(np.float64) @ w_v.astype(np.float64)).astype(np.float16)
    ulin = (FQS * ALPHA) * u.astype(np.float32)
    constB = np.zeros((128, 384), np.float16)
    constB[:, 0:128] = np.broadcast_to(ulin[0:128, None], (128, 128))
    constB[:, 128:256] = np.broadcast_to(ulin[128:256, None], (128, 128))
    constB[:, 256:384] = np.eye(128, dtype=np.float16)

    # f32 const slots at the tail of constA's c=0 plane
    a0, a1, a2 = [float(c) for c in COEFS]
    wv = FQS * w_v.astype(np.float64)
    fconsts = [np.full(128, -np.pi),                 # bnpi
               wv * a0, wv * a0,                     # (0,0) (0,1)
               wv * (-4 * a1), wv * (-4 * a1),       # (1,0) (1,1) scale
               wv * (-16 * a2), wv * (-16 * a2),     # (2,0) (2,1)
               wv * (2 * a1),                        # fbias1
               np.full(128, -0.5 * np.pi)]           # bnhpi
    cslots = np.zeros((128, 2, 32), np.float16)
    for i, v in enumerate(fconsts):
        cslots[:, 0, 2 * i : 2 * i + 2] = (
            np.asarray(v, np.float32).reshape(128, 1).view(np.float16)
        )

    in_maps = []
    for core in range(N_CORES):
        b, qh = divmod(core, 2)
        # keysT[p, khalf, c, k'] = keys[b][khalf*512+k', c*128+p]
        keysT = np.ascontiguousarray(
            keys[b].T.reshape(2, 128, 2, KH).transpose(1, 2, 0, 3)
        ).astype(np.float16)
        qT = np.ascontiguousarray(
            queries[b, qh * QL : (qh + 1) * QL, :].T.reshape(2, 128, QL)
            .transpose(1, 0, 2)
        ).astype(np.float16)
        constA = np.concatenate([wq0, qT, wk0, cslots], axis=2)
        valT = np.ascontiguousarray(
            values[b].T.reshape(KC, 128, D).transpose(1, 0, 2)
        ).astype(np.float16)
        in_maps.append(
            {
                "constA": np.ascontiguousarray(constA),
                "constB": constB,
                "keysT": keysT,
                "valT": valT,
            }
        )
    return in_maps


def _run(queries, keys, values, W_q, W_k, w_v, trace=False):
    global _nc_cache
    if _nc_cache is None:
        _nc_cache = build()
    nc = _nc_cache
    in_maps = _make_in_maps(queries, keys, values, W_q, W_k, w_v)
    res = run_bass_kernel_spmd(nc, in_maps, core_ids=list(range(N_CORES)), trace=trace)
    out = np.empty((B, Q, D), np.float32)
    for c in range(N_CORES):
        b, qh = divmod(c, 2)
        out[b, qh * QL : (qh + 1) * QL, :] = res.results[c]["out"].astype(np.float32)
    return out, res


def kernel(queries, keys, values, W_q, W_k, w_v):
    out, _ = _run(queries, keys, values, W_q, W_k, w_v, trace=False)
    return out


# revision 26
# speedup vs baseline: 1.0629x; 1.0629x over previous
"""Additive (Bahdanau) attention on 8 TRN2 NeuronCores, data-parallel.

Full problem: queries (4,256,256), keys (4,1024,256), values (4,256,1024),
W_q (256,128), W_k (256,128), w_v (128,) ->
    out[b,q,d] = softmax_k( sum_h w_v[h]*tanh((q W_q)[b,q,h]+(k W_k)[b,k,h]) ) @ values[b,d,:]^T

Sharding: 8 cores = (batch b, half of Q). Each core handles 128 queries with
its batch's full keys/values. No collectives needed.

Algorithm: the tanh feature tensor (Q*K*H elements) is never materialized.
tanh(s) ~= alpha*s + sum_r a_r sin(r*w*s)  (harmonics r=1..4 of one
fundamental): sin(w(x+y)) expands into separable sin/cos products, so scores
become PE matmuls with contraction H per (harmonic, trig) pair, plus a linear
term computed straight from keys with a host-precomputed broadcast stationary
u = W_k @ w_v.  Only the FUNDAMENTAL needs feature matmuls and ACT sins;
harmonics 2..4 are derived on DVE with double/triple/quadruple-angle algebra
in raw form (s1*c1, s1^2, (s1^2-.75)s1, ...), with the angle-formula
constants folded into the per-(r,t) Q-side fold scales and softmax-invariant
per-q constants dropped.

Range reduction for the ACT Sin spline ([-pi,pi] only), in "turns" units:
DVE computes n = round(v) via the fp32 magic constant, f = v - n, and the
cosine argument z = f + (f < 0.25) in one fused scalar_tensor_tensor,
consumed as sin(2*pi*z - 3*pi/2) = cos(2*pi*v)  (z in [0.25, 1.25)).
"""

import os
import sys
import types

import numpy as np

# ---------------------------------------------------------------------------
# antenv.axon_hooks shim: the image's antenv package lacks axon_hooks, which
# run_bass_kernel_spmd(trace=True) imports for NTFF profiling under axon.
if "antenv.axon_hooks" not in sys.modules:
    _m = types.ModuleType("antenv.axon_hooks")
    _m._hook = None
    _m.set_axon_ntff_profile_hook = lambda h: setattr(_m, "_hook", h)
    _m.get_axon_ntff_profile_hook = lambda: _m._hook
    sys.modules["antenv.axon_hooks"] = _m
    try:
        from trn_agent_boot.trn_boot import _ntff_profile_via_ctypes

        _m.set_axon_ntff_profile_hook(
            _ntff_profile_via_ctypes("/opt/axon/libaxon_pjrt.so")
        )
    except Exception:
        pass

import concourse.bass as bass
import concourse.tile as tile
from concourse import mybir
from concourse.bass_utils import run_bass_kernel_spmd
from concourse.vector_clock import ScopedClock

# ---------------------------------------------------------------------------
# This walrus build rejects >1 sync-wait command on one instruction; Tile's
# kernel-tail drain accumulates one wait per outstanding semaphore. Split the
# overflow onto follow-up SP nops.
_MAX_WAITS = 1


def _patched_drain_and_barrier(self, tick_clock, wait_clock):
    """Minimal kernel tail: drain with its sem waits only.  The NEFF is
    load->execute-once->unload per call, so the end-of-kernel barrier and
    semaphore cleanup (7+us of EVENT_SEMAPHORE ping-pong) are dead weight."""
    nc = self.nc
    drain_inst = nc.sync.drain()
    wait_clock.add_sem_waits(
        drain_inst.ins, ScopedClock({None: tick_clock.global_clock})
    )
    si = drain_inst.ins.sync_info
    if si is not None and len(si.on_wait) > _MAX_WAITS:
        waits = list(si.on_wait)
        drain_inst.ins.sync_info = mybir.SyncInfo(
            on_wait=waits[:_MAX_WAITS], on_update=list(si.on_update)
        )
        for k in range(_MAX_WAITS, len(waits), _MAX_WAITS):
            extra = nc.sync.nop()
            extra.ins.sync_info = mybir.SyncInfo(
                on_wait=waits[k : k + _MAX_WAITS], on_update=[]
            )
    assert self.sems is not None
    popped = nc._tile_sem_poison_stack.pop()
    assert popped is self._sem_poison


tile.TileContext._drain_and_barrier = _patched_drain_and_barrier

_nopctr = 0


def _split_multi_waits(nc, max_waits=_MAX_WAITS):
    """Walrus here allows only one sem-wait command per instruction; move
    extra waits onto preceding same-engine NOPs (semantically identical:
    the engine blocks on each wait in order)."""
    global _nopctr
    for f in nc.m.functions:
        for bb in f.blocks:
            insts = bb.instructions
            out = []
            changed = False
            for inst in insts:
                si = inst.sync_info
                if si is not None and len(si.on_wait) > max_waits:
                    changed = True
                    waits = list(si.on_wait)
                    n_extra = len(waits) - max_waits
                    for k in range(0, n_extra, max_waits):
                        nop = mybir.InstNoOp(name=f"waitsplit_{_nopctr}", ins=[], outs=[])
                        _nopctr += 1
                        nop.engine = inst.engine
                        nop.sync_info = mybir.SyncInfo(
                            on_wait=waits[k : min(k + max_waits, n_extra)], on_update=[]
                        )
                        out.append(nop)
                    inst.sync_info = mybir.SyncInfo(
                        on_wait=waits[n_extra:], on_update=list(si.on_update)
                    )
                out.append(inst)
            if changed:
                bb.instructions = out


# ---------------------------------------------------------------------------
B, Q, K = 4, 256, 1024
I, H, D = 256, 128, 256  # input dim, hidden dim, value dim
QL = 128  # queries per core
KH = 512  # K half
KC = K // 128  # 8 key chunks of 128
N_CORES = 8
R = 3  # number of harmonics
F32 = mybir.dt.float32
F16 = mybir.dt.float16
F8 = mybir.dt.float8e4

# Weighted LS fit of tanh(s) ~= ALPHA*s + sum_r COEFS[r]*sin((r+1)*W0*s)
# over s ~ N(0, 1.55); end-to-end rel err ~5e-3 (f16).
W0 = 0.8911
COEFS = np.array([0.4814, 0.1000, 0.0351], dtype=np.float64)
ALPHA = 0.2782
C1R0 = float(np.float32(W0 / (2 * np.pi)))  # "turns" scale, folded into W

SCORE_FP8 = False  # fp8e4 DoubleRow score matmuls (2x PE) at ~1e-2 rel err
FQS = 16.0 if SCORE_FP8 else 1.0  # Fq prescale (fp8 subnormal avoidance)
N_WARM = 12   # PE p-state warm-up matmuls before real work
N_FILL_B = 10  # fillers between vq and vk (cover keysT wait)
N_FILL_C = 8   # fillers before the score block (keep clock high)
N_FILL_D = 8   # fillers between the h0 and h1 score blocks
N_FILL_E = 8   # fillers during the exp phase (keep clock for out matmuls)

TPI = float(2 * np.pi)
PI = float(np.pi)
KMAG = float(np.float32(1.5 * 2**23))  # fp32 round-to-nearest magic constant
CAW = 416  # constA c-tile: 128 wq0 | 128 q | 128 wk0 | 32 f32-const slots

_nc_cache = None


def build():
    nc = bass.Bass("TRN2", target_bir_lowering=False, debug=False, num_devices=N_CORES)
    constA_ext = nc.declare_dram_parameter("constA", [128, 2, CAW], F16, isOutput=False)
    constB_ext = nc.declare_dram_parameter("constB", [128, 384], F16, isOutput=False)
    keysT_ext = nc.declare_dram_parameter("keysT", [128, 2, 2, KH], F16, isOutput=False)
    valT_ext = nc.declare_dram_parameter("valT", [128, KC, D], F16, isOutput=False)
    out_ext = nc.declare_dram_parameter("out", [QL, D], F16, isOutput=True)

    with tile.TileContext(nc) as tc:
        _build_body(nc, tc, constA_ext, constB_ext, keysT_ext, valT_ext, out_ext)
    _split_multi_waits(nc)
    return nc


def _build_body(nc, tc, constA_ext, constB_ext, keysT_ext, valT_ext, out_ext):
    A = mybir.AluOpType
    AF = mybir.ActivationFunctionType
    FD8 = F8 if SCORE_FP8 else F16
    with (
        tc.tile_pool(name="const", bufs=1) as constp,
        tc.tile_pool(name="big", bufs=1) as bigp,
        tc.tile_pool(name="wrk", bufs=2) as wrkp,
        tc.tile_pool(name="sc0p", bufs=1, space="PSUM") as sc0p,
        tc.tile_pool(name="sc1p", bufs=1, space="PSUM") as sc1p,
        tc.tile_pool(name="vkp", bufs=2, space="PSUM") as vkp,
        tc.tile_pool(name="tp0p", bufs=1, space="PSUM") as tp0p,
        tc.tile_pool(name="tp1p", bufs=1, space="PSUM") as tp1p,
        tc.tile_pool(name="o0p", bufs=1, space="PSUM") as o0p,
        tc.tile_pool(name="o1p", bufs=1, space="PSUM") as o1p,
    ):
        # ---- SBUF tiles --------------------------------------------------
        constA = constp.tile([128, 2, CAW], F16)  # wq0 | queriesT | wk0 | consts
        constB = constp.tile([128, 384], F16)     # ulin c0 | ulin c1 | ident
        keysT = bigp.tile([128, 2, 2, KH], F16)   # [p, khalf, c, k']
        valT = bigp.tile([128, KC, D], F16)
        scratch_sb = bigp.tile([128, 256], F16)
        argsq = bigp.tile([128, 2, QL], F16)      # plane1=f, plane0=z
        argsk = bigp.tile([128, 2, K], F16)       # plane1=f, plane0=z
        # Fqraw[r,t] = raw q-side factor for pair (r,t); Fq = scale*raw (+bias)
        Fqraw = bigp.tile([128, R, 2, QL], F16)
        Fq = bigp.tile([128, R, 2, QL], FD8)
        # G[r,t] = raw k-side partner of pair (r,t):
        #   r0: (c1k, s1k); r1: (Uk, Pk); r2: (p3ck, p3k)
        G = bigp.tile([128, R, 2, K], FD8)
        u2q = bigp.tile([128, QL], F16)
        attn0 = bigp.tile([128, KH], F16)
        attn1 = bigp.tile([128, KH], F16)
        attn_h = [attn0, attn1]
        attnT0 = bigp.tile([128, 4, QL], F16)
        attnT1 = bigp.tile([128, 4, QL], F16)
        attnT_h = [attnT0, attnT1]
        esum = bigp.tile([128, 2], F32)
        sums = bigp.tile([128, 1], F32)
        rs = bigp.tile([128, 1], F32)
        out_sb0 = bigp.tile([128, 128], F16)
        out_sb1 = bigp.tile([128, 128], F16)
        atl_dummy = bigp.tile([128, 1], F16)

        # ---- PSUM tiles (split per half so readers don't false-wait on
        # the sibling half's accumulation group).  vq_ps shares scores0's
        # bank: it is fully consumed before the ulin start=True resets it.
        vq_ps = sc0p.tile([128, KH], F32, tag="s0", name="vq_ps")
        scores0 = sc0p.tile([128, KH], F32, tag="s0", name="scores0")
        scores1 = sc1p.tile([128, KH], F32)
        scores_h = [scores0, scores1]
        tp0 = tp0p.tile([128, KH], F16)
        tp1 = tp1p.tile([128, KH], F16)
        tp_h = [tp0, tp1]
        out_ps0 = o0p.tile([128, 128], F32)           # also the filler target
        out_ps1 = o1p.tile([128, 128], F32)

        wq0 = lambda c: constA[:, c, 0:128]
        queriesT = lambda c: constA[:, c, 128:256]
        wk0 = lambda c: constA[:, c, 256:384]
        ulin = lambda c: constB[:, c * 128 : (c + 1) * 128]
        ident = constB[:, 256:384]
        fc = lambda i: constA[:, 0, 384 + 2 * i : 386 + 2 * i].bitcast(F32)
        bnpi = fc(0)                    # -pi
        fsc = lambda r, t: fc(1 + 2 * r + t)   # fold scales (r,t)
        fbias1 = fc(7)                  # fold bias for (1,1)
        bnhpi = fc(8)                   # -pi/2

        # ---- input DMAs: triggers spread across engines so descriptor
        # generation (~0.65us each) runs in parallel --------------------
        nc.sync.dma_start(out=constA[:], in_=constA_ext[:])
        nc.scalar.dma_start(out=keysT[:, 0:1], in_=keysT_ext[:, 0:1])
        nc.gpsimd.dma_start(out=keysT[:, 1:2], in_=keysT_ext[:, 1:2])
        nc.sync.dma_start(out=constB[:], in_=constB_ext[:])

        # ---- PE: p-state warm-up fillers during the DMA window -----------
        nc.gpsimd.memset(scratch_sb[:], 0)

        def fillers(n):
            for _ in range(n):
                nc.tensor.matmul(
                    out_ps0[:], scratch_sb[:, 0:128],
                    scratch_sb[:, 0:128], start=True, stop=True,
                )

        fillers(N_WARM)

        # ---- PE: vq, vk, ulin -------------------------------------------
        for c in range(2):
            nc.tensor.matmul(
                vq_ps[:, 0:QL], wq0(c), queriesT(c), start=(c == 0), stop=(c == 1)
            )
        fillers(N_FILL_B)
        vk_tiles = {}
        vk_insts = {}
        for h in range(2):
            vk_t = vkp.tile([128, KH], F32, tag="vk", name=f"vk_{h}")
            vk_tiles[h] = vk_t
            for c in range(2):
                vk_insts[(h, c)] = nc.tensor.matmul(
                    vk_t[:], wk0(c), keysT[:, h, c, :],
                    start=(c == 0), stop=(c == 1),
                )
        # linear term straight from keys: scores[q,k] += FQS*alpha*u.keys
        for h in range(2):
            for c in range(2):
                nc.tensor.matmul(
                    scores_h[h][:], ulin(c), keysT[:, h, c, :],
                    start=(c == 0), stop=False,
                )
        fillers(N_FILL_C)

        # ---- DVE: single-wrap args; |v| <= 0.70 < 1 so one conditional
        # +1 suffices: ys = (v<0)+v in [0,1), sin(2pi*ys - pi) = -sin(2pi*v);
        # yc = (v<-0.25)+v in [-0.25,0.75], sin(2pi*yc - pi/2) = -cos(2pi*v).
        # Every score pair has an even number of sign-flipped factors, so the
        # minus signs cancel identically.
        vqs = wrkp.tile([128, QL], F16, tag="vqs")
        nc.vector.tensor_copy(vqs[:], vq_ps[:, 0:QL])
        nc.vector.scalar_tensor_tensor(
            argsq[:, 1, :], vqs[:], 0.0, vqs[:], A.is_lt, A.add
        )
        nc.vector.scalar_tensor_tensor(
            argsq[:, 0, :], vqs[:], -0.25, vqs[:], A.is_lt, A.add
        )
        for h in range(2):
            lo, hi = h * KH, (h + 1) * KH
            vks = wrkp.tile([128, KH], F16, tag="vks", name=f"vks_{h}")
            nc.vector.tensor_copy(vks[:], vk_tiles[h][:])
            nc.vector.scalar_tensor_tensor(
                argsk[:, 1, lo:hi], vks[:], 0.0, vks[:], A.is_lt, A.add
            )
            nc.vector.scalar_tensor_tensor(
                argsk[:, 0, lo:hi], vks[:], -0.25, vks[:], A.is_lt, A.add
            )

        # values DMA: deferred so it doesn't steal HBM bandwidth at startup
        vdma = nc.gpsimd.dma_start(out=valT[:], in_=valT_ext[:])
        tile.add_dep_helper(
            vdma.ins, vk_insts[(1, 1)].ins,
            sync=True, reason="defer values DMA past startup",
        )

        # ---- ACT q sins, then DVE q derived raws -------------------------
        s1q, c1q = Fqraw[:, 0, 0, :], Fqraw[:, 0, 1, :]
        Pq, Uq = Fqraw[:, 1, 0, :], Fqraw[:, 1, 1, :]
        p3q, p3cq = Fqraw[:, 2, 0, :], Fqraw[:, 2, 1, :]
        nc.scalar.activation(s1q, argsq[:, 1, :], AF.Sin, scale=TPI, bias=bnpi)
        nc.scalar.activation(c1q, argsq[:, 0, :], AF.Sin, scale=TPI, bias=bnhpi)
        nc.vector.tensor_tensor(Pq, s1q, c1q, A.mult)
        nc.vector.tensor_tensor(Uq, s1q, s1q, A.mult)
        nc.vector.scalar_tensor_tensor(p3q, Uq, -0.75, s1q, A.add, A.mult)
        nc.vector.tensor_tensor(u2q[:], c1q, c1q, A.mult)
        nc.vector.scalar_tensor_tensor(p3cq, u2q[:], -0.75, c1q, A.add, A.mult)
        # DVE folds (pure scales); the affine (1,1) fold runs on ACT below
        for r, t in [(0, 0), (0, 1), (1, 0), (2, 0), (2, 1)]:
            nc.vector.tensor_scalar_mul(
                Fq[:, r, t, :], Fqraw[:, r, t, :], fsc(r, t)
            )

        # ---- per K-half: ACT sins+squares, DVE products ------------------
        for h in range(2):
            lo, hi = h * KH, (h + 1) * KH
            c1k, s1k = G[:, 0, 0, lo:hi], G[:, 0, 1, lo:hi]
            Uk, Pk = G[:, 1, 0, lo:hi], G[:, 1, 1, lo:hi]
            p3ck, p3k = G[:, 2, 0, lo:hi], G[:, 2, 1, lo:hi]
            nc.scalar.activation(s1k, argsk[:, 1, lo:hi], AF.Sin, scale=TPI, bias=bnpi)
            nc.scalar.activation(
                c1k, argsk[:, 0, lo:hi], AF.Sin, scale=TPI, bias=bnhpi
            )
            if h == 1:
                # dummy exp right after the LAST Sin: hoists the exp-table
                # load here; Square/Identity exist in both table sets.
                nc.scalar.activation(atl_dummy[:], scratch_sb[:, 0:1], AF.Exp)
            nc.scalar.activation(Uk, s1k, AF.Square)
            u2k = wrkp.tile([128, KH], F16, tag="u2k", name=f"u2k_{h}")
            nc.vector.tensor_tensor(u2k[:], c1k, c1k, A.mult)
            if h == 0:
                nc.scalar.activation(
                    Fq[:, 1, 1, :], Fqraw[:, 1, 1, :], AF.Identity,
                    scale=fsc(1, 1), bias=fbias1,
                )
            nc.vector.tensor_tensor(Pk, s1k, c1k, A.mult)
            nc.vector.scalar_tensor_tensor(p3k, Uk, -0.75, s1k, A.add, A.mult)
            nc.vector.scalar_tensor_tensor(p3ck, u2k[:], -0.75, c1k, A.add, A.mult)

        # ---- PE score matmuls: h0 block strictly before h1 block ---------
        for h in range(2):
            lo, hi = h * KH, (h + 1) * KH
            if h == 1:
                fillers(N_FILL_D)
            if SCORE_FP8:
                for r in range(R):
                    nc.tensor.matmul(
                        scores_h[h][:], Fq[:, r, :, :], G[:, r, :, lo:hi],
                        start=False, stop=(r == R - 1),
                        perf_mode=mybir.MatmulPerfMode.DoubleRow,
                    )
            else:
                for r in range(R):
                    for t in range(2):
                        nc.tensor.matmul(
                            scores_h[h][:], Fq[:, r, t, :], G[:, r, t, lo:hi],
                            start=False, stop=(r == R - 1 and t == 1),
                        )

        # ---- softmax + attn^T + out, pipelined per K-half ----------------
        fillers(N_FILL_E)
        for h in range(2):
            nc.scalar.activation(
                attn_h[h][:], scores_h[h][:], AF.Exp, scale=1.0 / FQS,
                accum_out=esum[:, h : h + 1],
            )
            for j in range(4):
                nc.tensor.transpose(
                    tp_h[h][:, j * 128 : (j + 1) * 128],
                    attn_h[h][:, j * 128 : (j + 1) * 128],
                    ident,
                )
            nc.vector.tensor_copy(attnT_h[h][:], tp_h[h][:])
            if h == 0:
                for j in range(4):
                    for dg, ops in ((0, out_ps0), (1, out_ps1)):
                        nc.tensor.matmul(
                            ops[:], attnT_h[0][:, j, :],
                            valT[:, j, dg * 128 : (dg + 1) * 128],
                            start=(j == 0), stop=False,
                        )
        # h1: finish the d0 accumulation first so its scale+DMA (descriptor
        # generation ~0.6us) overlaps the d1 matmuls
        nc.vector.tensor_tensor(sums[:], esum[:, 0:1], esum[:, 1:2], A.add)
        nc.vector.reciprocal(rs[:], sums[:])
        for j in range(4):
            nc.tensor.matmul(
                out_ps0[:], attnT_h[1][:, j, :], valT[:, 4 + j, 0:128],
                start=False, stop=(j == 3),
            )
        nc.vector.tensor_scalar_mul(out_sb0[:], out_ps0[:], rs[:])
        nc.sync.dma_start(out=out_ext[:, 0:128], in_=out_sb0[:])
        for j in range(4):
            nc.tensor.matmul(
                out_ps1[:], attnT_h[1][:, j, :], valT[:, 4 + j, 128:256],
                start=False, stop=(j == 3),
            )
        nc.vector.tensor_scalar_mul(out_sb1[:], out_ps1[:], rs[:])
        dbg = os.environ.get("KDBG", "")
        if dbg == "attn":
            nc.sync.dma_start(out=out_ext[:], in_=attn0[:, 0:256])
        elif dbg == "argf":
            nc.sync.dma_start(out=out_ext[:], in_=argsk[:, 1, 0:256])
        elif dbg == "argz":
            nc.sync.dma_start(out=out_ext[:], in_=argsk[:, 0, 0:256])
        elif dbg == "s1k":
            nc.sync.dma_start(out=out_ext[:], in_=G[:, 0, 1, 0:256])
        elif dbg == "c1k":
            nc.sync.dma_start(out=out_ext[:], in_=G[:, 0, 0, 0:256])
        elif dbg == "pk":
            nc.sync.dma_start(out=out_ext[:], in_=G[:, 1, 1, 0:256])
        elif dbg == "fq":
            nc.sync.dma_start(out=out_ext[:], in_=Fq[:, 0, :, :])
        elif dbg == "fqraw":
            nc.sync.dma_start(out=out_ext[:], in_=Fqraw[:, 0, :, :])
        elif dbg == "uk":
            nc.sync.dma_start(out=out_ext[:], in_=G[:, 1, 0, 0:256])
        else:
            nc.sync.dma_start(out=out_ext[:, 128:256], in_=out_sb1[:])


def _make_in_maps(queries, keys, values, W_q, W_k, w_v):
    queries = np.asarray(queries, np.float32)
    keys = np.asarray(keys, np.float32)
    values = np.asarray(values, np.float32)
    W_q = np.asarray(W_q, np.float32)
    W_k = np.asarray(W_k, np.float32)
    w_v = np.asarray(w_v, np.float32).reshape(H)

    # fundamental-only stationaries in "turns": wq0[p,c,h] = W_q[c*128+p,h]*C1R0
    wq0 = np.ascontiguousarray(
        (W_q * C1R0).reshape(2, 128, H).transpose(1, 0, 2)
    ).astype(np.float16)
    wk0 = np.ascontiguousarray(
        (W_k * C1R0).reshape(2, 128, H).transpose(1, 0, 2)
    ).astype(np.float16)

    u = (W_k.astype(np.float64) @ w_v.astype(np.float64)).astype(np.float16)
    ulin = (FQS * ALPHA) * u.astype(np.float32)
    constB = np.zeros((128, 384), np.float16)
    constB[:, 0:128] = np.broadcast_to(ulin[0:128, None], (128, 128))
    constB[:, 128:256] = np.broadcast_to(ulin[128:256, None], (128, 128))
    constB[:, 256:384] = np.eye(128, dtype=np.float16)

    # f32 const slots at the tail of constA's c=0 plane
    a0, a1, a2 = [float(c) for c in COEFS]
    wv = FQS * w_v.astype(np.float64)
    fconsts = [np.full(128, -np.pi),                 # bnpi
               wv * a0, wv * a0,                     # (0,0) (0,1)
               wv * (-4 * a1), wv * (-4 * a1),       # (1,0) (1,1) scale
               wv * (-16 * a2), wv * (-16 * a2),     # (2,0) (2,1)
               wv * (2 * a1),                        # fbias1
               np.full(128, -0.5 * np.pi)]           # bnhpi
    cslots = np.zeros((128, 2, 32), np.float16)
    for i, v in enumerate(fconsts):
        cslots[:, 0, 2 * i : 2 * i + 2] = (
            np.asarray(v, np.float32).reshape(128, 1).view(np.float16)
        )

    in_maps = []
    for core in range(N_CORES):
        b, qh = divmod(core, 2)
        # keysT[p, khalf, c, k'] = keys[b][khalf*512+k', c*128+p]
        keysT = np.ascontiguousarray(
            keys[b].T.reshape(2, 128, 2, KH).transpose(1, 2, 0, 3)
        ).astype(np.float16)
        qT = np.ascontiguousarray(
            queries[b, qh * QL : (qh + 1) * QL, :].T.reshape(2, 128, QL)
            .transpose(1, 0, 2)
        ).astype(np.float16)
        constA = np.concatenate([wq0, qT, wk0, cslots], axis=2)
        valT = np.ascontiguousarray(
            values[b].T.reshape(KC, 128, D).transpose(1, 0, 2)
        ).astype(np.float16)
        in_maps.append(
            {
                "constA": np.ascontiguousarray(constA),
                "constB": constB,
                "keysT": keysT,
                "valT": valT,
            }
        )
    return in_maps


def _run(queries, keys, values, W_q, W_k, w_v, trace=False):
    global _nc_cache
    if _nc_cache is None:
        _nc_cache = build()
    nc = _nc_cache
    in_maps = _make_in_maps(queries, keys, values, W_q, W_k, w_v)
    res = run_bass_kernel_spmd(nc, in_maps, core_ids=list(range(N_CORES)), trace=trace)
    out = np.empty((B, Q, D), np.float32)
    for c in range(N_CORES):
        b, qh = divmod(c, 2)
        out[b, qh * QL : (qh + 1) * QL, :] = res.results[c]["out"].astype(np.float32)
    return out, res


def kernel(queries, keys, values, W_q, W_k, w_v):
    out, _ = _run(queries, keys, values, W_q, W_k, w_v, trace=False)
    return out


# revision 27
# speedup vs baseline: 1.0696x; 1.0063x over previous
"""Additive (Bahdanau) attention on 8 TRN2 NeuronCores, data-parallel.

Full problem: queries (4,256,256), keys (4,1024,256), values (4,256,1024),
W_q (256,128), W_k (256,128), w_v (128,) ->
    out[b,q,d] = softmax_k( sum_h w_v[h]*tanh((q W_q)[b,q,h]+(k W_k)[b,k,h]) ) @ values[b,d,:]^T

Sharding: 8 cores = (batch b, half of Q). Each core handles 128 queries with
its batch's full keys/values. No collectives needed.

Algorithm: the tanh feature tensor (Q*K*H elements) is never materialized.
tanh(s) ~= alpha*s + sum_r a_r sin(r*w*s)  (harmonics r=1..4 of one
fundamental): sin(w(x+y)) expands into separable sin/cos products, so scores
become PE matmuls with contraction H per (harmonic, trig) pair, plus a linear
term computed straight from keys with a host-precomputed broadcast stationary
u = W_k @ w_v.  Only the FUNDAMENTAL needs feature matmuls and ACT sins;
harmonics 2..4 are derived on DVE with double/triple/quadruple-angle algebra
in raw form (s1*c1, s1^2, (s1^2-.75)s1, ...), with the angle-formula
constants folded into the per-(r,t) Q-side fold scales and softmax-invariant
per-q constants dropped.

Range reduction for the ACT Sin spline ([-pi,pi] only), in "turns" units:
DVE computes n = round(v) via the fp32 magic constant, f = v - n, and the
cosine argument z = f + (f < 0.25) in one fused scalar_tensor_tensor,
consumed as sin(2*pi*z - 3*pi/2) = cos(2*pi*v)  (z in [0.25, 1.25)).
"""

import os
import sys
import types

import numpy as np

# ---------------------------------------------------------------------------
# antenv.axon_hooks shim: the image's antenv package lacks axon_hooks, which
# run_bass_kernel_spmd(trace=True) imports for NTFF profiling under axon.
if "antenv.axon_hooks" not in sys.modules:
    _m = types.ModuleType("antenv.axon_hooks")
    _m._hook = None
    _m.set_axon_ntff_profile_hook = lambda h: setattr(_m, "_hook", h)
    _m.get_axon_ntff_profile_hook = lambda: _m._hook
    sys.modules["antenv.axon_hooks"] = _m
    try:
        from trn_agent_boot.trn_boot import _ntff_profile_via_ctypes

        _m.set_axon_ntff_profile_hook(
            _ntff_profile_via_ctypes("/opt/axon/libaxon_pjrt.so")
        )
    except Exception:
        pass

import concourse.bass as bass
import concourse.tile as tile
from concourse import mybir
from concourse.bass_utils import run_bass_kernel_spmd
from concourse.vector_clock import ScopedClock

# ---------------------------------------------------------------------------
# This walrus build rejects >1 sync-wait command on one instruction; Tile's
# kernel-tail drain accumulates one wait per outstanding semaphore. Split the
# overflow onto follow-up SP nops.
_MAX_WAITS = 1


def _patched_drain_and_barrier(self, tick_clock, wait_clock):
    """Minimal kernel tail: drain with its sem waits only.  The NEFF is
    load->execute-once->unload per call, so the end-of-kernel barrier and
    semaphore cleanup (7+us of EVENT_SEMAPHORE ping-pong) are dead weight."""
    nc = self.nc
    drain_inst = nc.sync.drain()
    wait_clock.add_sem_waits(
        drain_inst.ins, ScopedClock({None: tick_clock.global_clock})
    )
    si = drain_inst.ins.sync_info
    if si is not None and len(si.on_wait) > _MAX_WAITS:
        waits = list(si.on_wait)
        drain_inst.ins.sync_info = mybir.SyncInfo(
            on_wait=waits[:_MAX_WAITS], on_update=list(si.on_update)
        )
        for k in range(_MAX_WAITS, len(waits), _MAX_WAITS):
            extra = nc.sync.nop()
            extra.ins.sync_info = mybir.SyncInfo(
                on_wait=waits[k : k + _MAX_WAITS], on_update=[]
            )
    assert self.sems is not None
    popped = nc._tile_sem_poison_stack.pop()
    assert popped is self._sem_poison


tile.TileContext._drain_and_barrier = _patched_drain_and_barrier

_nopctr = 0


def _split_multi_waits(nc, max_waits=_MAX_WAITS):
    """Walrus here allows only one sem-wait command per instruction; move
    extra waits onto preceding same-engine NOPs (semantically identical:
    the engine blocks on each wait in order)."""
    global _nopctr
    for f in nc.m.functions:
        for bb in f.blocks:
            insts = bb.instructions
            out = []
            changed = False
            for inst in insts:
                si = inst.sync_info
                if si is not None and len(si.on_wait) > max_waits:
                    changed = True
                    waits = list(si.on_wait)
                    n_extra = len(waits) - max_waits
                    for k in range(0, n_extra, max_waits):
                        nop = mybir.InstNoOp(name=f"waitsplit_{_nopctr}", ins=[], outs=[])
                        _nopctr += 1
                        nop.engine = inst.engine
                        nop.sync_info = mybir.SyncInfo(
                            on_wait=waits[k : min(k + max_waits, n_extra)], on_update=[]
                        )
                        out.append(nop)
                    inst.sync_info = mybir.SyncInfo(
                        on_wait=waits[n_extra:], on_update=list(si.on_update)
                    )
                out.append(inst)
            if changed:
                bb.instructions = out


# ---------------------------------------------------------------------------
B, Q, K = 4, 256, 1024
I, H, D = 256, 128, 256  # input dim, hidden dim, value dim
QL = 128  # queries per core
KH = 512  # K half
KC = K // 128  # 8 key chunks of 128
N_CORES = 8
R = 3  # number of harmonics
F32 = mybir.dt.float32
F16 = mybir.dt.float16
F8 = mybir.dt.float8e4

# Weighted LS fit of tanh(s) ~= ALPHA*s + sum_r COEFS[r]*sin((r+1)*W0*s)
# over s ~ N(0, 1.55); end-to-end rel err ~5e-3 (f16).
W0 = 0.8911
COEFS = np.array([0.4814, 0.1000, 0.0351], dtype=np.float64)
ALPHA = 0.2782
C1R0 = float(np.float32(W0 / (2 * np.pi)))  # "turns" scale, folded into W

SCORE_FP8 = False  # fp8e4 DoubleRow score matmuls (2x PE) at ~1e-2 rel err
FQS = 16.0 if SCORE_FP8 else 1.0  # Fq prescale (fp8 subnormal avoidance)
N_WARM = 12   # PE p-state warm-up matmuls before real work
N_FILL_B = 10  # fillers between vq and vk (cover keysT wait)
N_FILL_C = 8   # fillers before the score block (keep clock high)
N_FILL_D = 8   # fillers between the h0 and h1 score blocks
N_FILL_E = 8   # fillers during the exp phase (keep clock for out matmuls)

TPI = float(2 * np.pi)
PI = float(np.pi)
KMAG = float(np.float32(1.5 * 2**23))  # fp32 round-to-nearest magic constant
CAW = 288  # constA c-tile: 128 wq0 | 128 q | 32 f32-const slots

_nc_cache = None


def build():
    nc = bass.Bass("TRN2", target_bir_lowering=False, debug=False, num_devices=N_CORES)
    constA_ext = nc.declare_dram_parameter("constA", [128, 2, CAW], F16, isOutput=False)
    constB_ext = nc.declare_dram_parameter("constB", [128, 384], F16, isOutput=False)
    keysT_ext = nc.declare_dram_parameter("keysT", [128, 2, 2, KH + H], F16, isOutput=False)
    valT_ext = nc.declare_dram_parameter("valT", [128, KC, D], F16, isOutput=False)
    out_ext = nc.declare_dram_parameter("out", [QL, D], F16, isOutput=True)

    with tile.TileContext(nc) as tc:
        _build_body(nc, tc, constA_ext, constB_ext, keysT_ext, valT_ext, out_ext)
    _split_multi_waits(nc)
    return nc


def _build_body(nc, tc, constA_ext, constB_ext, keysT_ext, valT_ext, out_ext):
    A = mybir.AluOpType
    AF = mybir.ActivationFunctionType
    FD8 = F8 if SCORE_FP8 else F16
    with (
        tc.tile_pool(name="const", bufs=1) as constp,
        tc.tile_pool(name="big", bufs=1) as bigp,
        tc.tile_pool(name="wrk", bufs=2) as wrkp,
        tc.tile_pool(name="sc0p", bufs=1, space="PSUM") as sc0p,
        tc.tile_pool(name="sc1p", bufs=1, space="PSUM") as sc1p,
        tc.tile_pool(name="vkp", bufs=2, space="PSUM") as vkp,
        tc.tile_pool(name="tp0p", bufs=1, space="PSUM") as tp0p,
        tc.tile_pool(name="tp1p", bufs=1, space="PSUM") as tp1p,
        tc.tile_pool(name="o0p", bufs=1, space="PSUM") as o0p,
        tc.tile_pool(name="o1p", bufs=1, space="PSUM") as o1p,
    ):
        # ---- SBUF tiles --------------------------------------------------
        constA = constp.tile([128, 2, CAW], F16)  # wq0 | queriesT | wk0 | consts
        constB = constp.tile([128, 384], F16)     # ulin c0 | ulin c1 | ident
        keysT = bigp.tile([128, 2, 2, KH + H], F16)  # [p, khalf, c, k'|wk0]
        valT = bigp.tile([128, KC, D], F16)
        scratch_sb = bigp.tile([128, 256], F16)
        argsq = bigp.tile([128, 2, QL], F16)      # plane1=f, plane0=z
        argsk = bigp.tile([128, 2, K], F16)       # plane1=f, plane0=z
        # Fqraw[r,t] = raw q-side factor for pair (r,t); Fq = scale*raw (+bias)
        Fqraw = bigp.tile([128, R, 2, QL], F16)
        Fq = bigp.tile([128, R, 2, QL], FD8)
        # G[r,t] = raw k-side partner of pair (r,t):
        #   r0: (c1k, s1k); r1: (Uk, Pk); r2: (p3ck, p3k)
        G = bigp.tile([128, R, 2, K], FD8)
        u2q = bigp.tile([128, QL], F16)
        attn0 = bigp.tile([128, KH], F16)
        attn1 = bigp.tile([128, KH], F16)
        attn_h = [attn0, attn1]
        attnT0 = bigp.tile([128, 4, QL], F16)
        attnT1 = bigp.tile([128, 4, QL], F16)
        attnT_h = [attnT0, attnT1]
        esum = bigp.tile([128, 2], F32)
        sums = bigp.tile([128, 1], F32)
        rs = bigp.tile([128, 1], F32)
        out_sb0 = bigp.tile([128, 128], F16)
        out_sb1 = bigp.tile([128, 128], F16)
        atl_dummy = bigp.tile([128, 1], F16)

        # ---- PSUM tiles (split per half so readers don't false-wait on
        # the sibling half's accumulation group).  vq_ps shares scores0's
        # bank: it is fully consumed before the ulin start=True resets it.
        vq_ps = sc0p.tile([128, KH], F32, tag="s0", name="vq_ps")
        scores0 = sc0p.tile([128, KH], F32, tag="s0", name="scores0")
        scores1 = sc1p.tile([128, KH], F32)
        scores_h = [scores0, scores1]
        tp0 = tp0p.tile([128, KH], F16)
        tp1 = tp1p.tile([128, KH], F16)
        tp_h = [tp0, tp1]
        out_ps0 = o0p.tile([128, 128], F32)           # also the filler target
        out_ps1 = o1p.tile([128, 128], F32)

        wq0 = lambda c: constA[:, c, 0:128]
        queriesT = lambda c: constA[:, c, 128:256]
        wk0 = lambda c: keysT[:, 0, c, KH : KH + H]
        ulin = lambda c: constB[:, c * 128 : (c + 1) * 128]
        ident = constB[:, 256:384]
        fc = lambda i: constA[:, 0, 256 + 2 * i : 258 + 2 * i].bitcast(F32)
        bnpi = fc(0)                    # -pi
        fsc = lambda r, t: fc(1 + 2 * r + t)   # fold scales (r,t)
        fbias1 = fc(7)                  # fold bias for (1,1)
        bnhpi = fc(8)                   # -pi/2

        # ---- input DMAs: triggers spread across engines so descriptor
        # generation (~0.65us each) runs in parallel --------------------
        nc.sync.dma_start(out=keysT[:, 0:1], in_=keysT_ext[:, 0:1])
        nc.scalar.dma_start(out=constA[:], in_=constA_ext[:])
        nc.sync.dma_start(out=keysT[:, 1:2], in_=keysT_ext[:, 1:2])
        nc.sync.dma_start(out=constB[:], in_=constB_ext[:])

        # ---- PE: p-state warm-up fillers during the DMA window -----------
        nc.gpsimd.memset(scratch_sb[:], 0)

        def fillers(n):
            for _ in range(n):
                nc.tensor.matmul(
                    out_ps0[:], scratch_sb[:, 0:128],
                    scratch_sb[:, 0:128], start=True, stop=True,
                )

        fillers(N_WARM)

        # ---- PE: vq, vk, ulin -------------------------------------------
        for c in range(2):
            nc.tensor.matmul(
                vq_ps[:, 0:QL], wq0(c), queriesT(c), start=(c == 0), stop=(c == 1)
            )
        fillers(N_FILL_B)
        vk_tiles = {}
        vk_insts = {}
        for h in range(2):
            vk_t = vkp.tile([128, KH], F32, tag="vk", name=f"vk_{h}")
            vk_tiles[h] = vk_t
            for c in range(2):
                vk_insts[(h, c)] = nc.tensor.matmul(
                    vk_t[:], wk0(c), keysT[:, h, c, 0:KH],
                    start=(c == 0), stop=(c == 1),
                )
        # linear term straight from keys: scores[q,k] += FQS*alpha*u.keys
        for h in range(2):
            for c in range(2):
                nc.tensor.matmul(
                    scores_h[h][:], ulin(c), keysT[:, h, c, 0:KH],
                    start=(c == 0), stop=False,
                )
        fillers(N_FILL_C)

        # ---- DVE: single-wrap args; |v| <= 0.70 < 1 so one conditional
        # +1 suffices: ys = (v<0)+v in [0,1), sin(2pi*ys - pi) = -sin(2pi*v);
        # yc = (v<-0.25)+v in [-0.25,0.75], sin(2pi*yc - pi/2) = -cos(2pi*v).
        # Every score pair has an even number of sign-flipped factors, so the
        # minus signs cancel identically.
        vqs = wrkp.tile([128, QL], F16, tag="vqs")
        nc.vector.tensor_copy(vqs[:], vq_ps[:, 0:QL])
        nc.vector.scalar_tensor_tensor(
            argsq[:, 1, :], vqs[:], 0.0, vqs[:], A.is_lt, A.add
        )
        nc.vector.scalar_tensor_tensor(
            argsq[:, 0, :], vqs[:], -0.25, vqs[:], A.is_lt, A.add
        )
        for h in range(2):
            lo, hi = h * KH, (h + 1) * KH
            vks = wrkp.tile([128, KH], F16, tag="vks", name=f"vks_{h}")
            nc.vector.tensor_copy(vks[:], vk_tiles[h][:])
            nc.vector.scalar_tensor_tensor(
                argsk[:, 1, lo:hi], vks[:], 0.0, vks[:], A.is_lt, A.add
            )
            nc.vector.scalar_tensor_tensor(
                argsk[:, 0, lo:hi], vks[:], -0.25, vks[:], A.is_lt, A.add
            )

        # values DMA: deferred so it doesn't steal HBM bandwidth at startup
        vdma = nc.gpsimd.dma_start(out=valT[:], in_=valT_ext[:])
        tile.add_dep_helper(
            vdma.ins, vk_insts[(1, 1)].ins,
            sync=True, reason="defer values DMA past startup",
        )

        # ---- ACT q sins, then DVE q derived raws -------------------------
        s1q, c1q = Fqraw[:, 0, 0, :], Fqraw[:, 0, 1, :]
        Pq, Uq = Fqraw[:, 1, 0, :], Fqraw[:, 1, 1, :]
        p3q, p3cq = Fqraw[:, 2, 0, :], Fqraw[:, 2, 1, :]
        nc.scalar.activation(s1q, argsq[:, 1, :], AF.Sin, scale=TPI, bias=bnpi)
        nc.scalar.activation(c1q, argsq[:, 0, :], AF.Sin, scale=TPI, bias=bnhpi)
        nc.vector.tensor_tensor(Pq, s1q, c1q, A.mult)
        nc.vector.tensor_tensor(Uq, s1q, s1q, A.mult)
        nc.vector.scalar_tensor_tensor(p3q, Uq, -0.75, s1q, A.add, A.mult)
        nc.vector.tensor_tensor(u2q[:], c1q, c1q, A.mult)
        nc.vector.scalar_tensor_tensor(p3cq, u2q[:], -0.75, c1q, A.add, A.mult)
        # DVE folds (pure scales); the affine (1,1) fold runs on ACT below
        for r, t in [(0, 0), (0, 1), (1, 0), (2, 0), (2, 1)]:
            nc.vector.tensor_scalar_mul(
                Fq[:, r, t, :], Fqraw[:, r, t, :], fsc(r, t)
            )

        # ---- per K-half: ACT sins+squares, DVE products ------------------
        for h in range(2):
            lo, hi = h * KH, (h + 1) * KH
            c1k, s1k = G[:, 0, 0, lo:hi], G[:, 0, 1, lo:hi]
            Uk, Pk = G[:, 1, 0, lo:hi], G[:, 1, 1, lo:hi]
            p3ck, p3k = G[:, 2, 0, lo:hi], G[:, 2, 1, lo:hi]
            nc.scalar.activation(s1k, argsk[:, 1, lo:hi], AF.Sin, scale=TPI, bias=bnpi)
            ck_sin = nc.scalar.activation(
                c1k, argsk[:, 0, lo:hi], AF.Sin, scale=TPI, bias=bnhpi
            )
            if h == 1:
                # dummy exp right after the LAST Sin: hoists the exp-table
                # load here; Square/Identity exist in both table sets. Pinned
                # so the scheduler cannot float it ahead of the sins.
                dexp = nc.scalar.activation(atl_dummy[:], scratch_sb[:, 0:1], AF.Exp)
                tile.add_dep_helper(
                    dexp.ins, ck_sin.ins, sync=False,
                    reason="exp-table load must follow the last sin",
                )
            nc.scalar.activation(Uk, s1k, AF.Square)
            u2k = wrkp.tile([128, KH], F16, tag="u2k", name=f"u2k_{h}")
            nc.vector.tensor_tensor(u2k[:], c1k, c1k, A.mult)
            if h == 0:
                nc.scalar.activation(
                    Fq[:, 1, 1, :], Fqraw[:, 1, 1, :], AF.Identity,
                    scale=fsc(1, 1), bias=fbias1,
                )
            nc.vector.tensor_tensor(Pk, s1k, c1k, A.mult)
            nc.vector.scalar_tensor_tensor(p3k, Uk, -0.75, s1k, A.add, A.mult)
            nc.vector.scalar_tensor_tensor(p3ck, u2k[:], -0.75, c1k, A.add, A.mult)

        # ---- PE score matmuls: h0 block strictly before h1 block ---------
        for h in range(2):
            lo, hi = h * KH, (h + 1) * KH
            if h == 1:
                fillers(N_FILL_D)
            if SCORE_FP8:
                for r in range(R):
                    nc.tensor.matmul(
                        scores_h[h][:], Fq[:, r, :, :], G[:, r, :, lo:hi],
                        start=False, stop=(r == R - 1),
                        perf_mode=mybir.MatmulPerfMode.DoubleRow,
                    )
            else:
                for r in range(R):
                    for t in range(2):
                        nc.tensor.matmul(
                            scores_h[h][:], Fq[:, r, t, :], G[:, r, t, lo:hi],
                            start=False, stop=(r == R - 1 and t == 1),
                        )

        # ---- softmax + attn^T + out, pipelined per K-half ----------------
        fillers(N_FILL_E)
        for h in range(2):
            nc.scalar.activation(
                attn_h[h][:], scores_h[h][:], AF.Exp, scale=1.0 / FQS,
                accum_out=esum[:, h : h + 1],
            )
            for j in range(4):
                nc.tensor.transpose(
                    tp_h[h][:, j * 128 : (j + 1) * 128],
                    attn_h[h][:, j * 128 : (j + 1) * 128],
                    ident,
                )
            nc.vector.tensor_copy(attnT_h[h][:], tp_h[h][:])
            if h == 0:
                for j in range(4):
                    for dg, ops in ((0, out_ps0), (1, out_ps1)):
                        nc.tensor.matmul(
                            ops[:], attnT_h[0][:, j, :],
                            valT[:, j, dg * 128 : (dg + 1) * 128],
                            start=(j == 0), stop=False,
                        )
        # h1: finish the d0 accumulation first so its scale+DMA (descriptor
        # generation ~0.6us) overlaps the d1 matmuls
        nc.vector.tensor_tensor(sums[:], esum[:, 0:1], esum[:, 1:2], A.add)
        nc.vector.reciprocal(rs[:], sums[:])
        for j in range(4):
            nc.tensor.matmul(
                out_ps0[:], attnT_h[1][:, j, :], valT[:, 4 + j, 0:128],
                start=False, stop=(j == 3),
            )
        nc.vector.tensor_scalar_mul(out_sb0[:], out_ps0[:], rs[:])
        nc.sync.dma_start(out=out_ext[:, 0:128], in_=out_sb0[:])
        for j in range(4):
            nc.tensor.matmul(
                out_ps1[:], attnT_h[1][:, j, :], valT[:, 4 + j, 128:256],
                start=False, stop=(j == 3),
            )
        nc.vector.tensor_scalar_mul(out_sb1[:], out_ps1[:], rs[:])
        dbg = os.environ.get("KDBG", "")
        if dbg == "attn":
            nc.sync.dma_start(out=out_ext[:], in_=attn0[:, 0:256])
        elif dbg == "argf":
            nc.sync.dma_start(out=out_ext[:], in_=argsk[:, 1, 0:256])
        elif dbg == "argz":
            nc.sync.dma_start(out=out_ext[:], in_=argsk[:, 0, 0:256])
        elif dbg == "s1k":
            nc.sync.dma_start(out=out_ext[:], in_=G[:, 0, 1, 0:256])
        elif dbg == "c1k":
            nc.sync.dma_start(out=out_ext[:], in_=G[:, 0, 0, 0:256])
        elif dbg == "pk":
            nc.sync.dma_start(out=out_ext[:], in_=G[:, 1, 1, 0:256])
        elif dbg == "fq":
            nc.sync.dma_start(out=out_ext[:], in_=Fq[:, 0, :, :])
        elif dbg == "fqraw":
            nc.sync.dma_start(out=out_ext[:], in_=Fqraw[:, 0, :, :])
        elif dbg == "uk":
            nc.sync.dma_start(out=out_ext[:], in_=G[:, 1, 0, 0:256])
        else:
            nc.sync.dma_start(out=out_ext[:, 128:256], in_=out_sb1[:])


def _make_in_maps(queries, keys, values, W_q, W_k, w_v):
    queries = np.asarray(queries, np.float32)
    keys = np.asarray(keys, np.float32)
    values = np.asarray(values, np.float32)
    W_q = np.asarray(W_q, np.float32)
    W_k = np.asarray(W_k, np.float32)
    w_v = np.asarray(w_v, np.float32).reshape(H)

    # fundamental-only stationaries in "turns": wq0[p,c,h] = W_q[c*128+p,h]*C1R0
    wq0 = np.ascontiguousarray(
        (W_q * C1R0).reshape(2, 128, H).transpose(1, 0, 2)
    ).astype(np.float16)
    wk0 = np.ascontiguousarray(
        (W_k * C1R0).reshape(2, 128, H).transpose(1, 0, 2)
    ).astype(np.float16)

    u = (W_k.astype(np.float64) @ w_v.astype(np.float64)).astype(np.float16)
    ulin = (FQS * ALPHA) * u.astype(np.float32)
    constB = np.zeros((128, 384), np.float16)
    constB[:, 0:128] = np.broadcast_to(ulin[0:128, None], (128, 128))
    constB[:, 128:256] = np.broadcast_to(ulin[128:256, None], (128, 128))
    constB[:, 256:384] = np.eye(128, dtype=np.float16)

    # f32 const slots at the tail of constA's c=0 plane
    a0, a1, a2 = [float(c) for c in COEFS]
    wv = FQS * w_v.astype(np.float64)
    fconsts = [np.full(128, -np.pi),                 # bnpi
               wv * a0, wv * a0,                     # (0,0) (0,1)
               wv * (-4 * a1), wv * (-4 * a1),       # (1,0) (1,1) scale
               wv * (-16 * a2), wv * (-16 * a2),     # (2,0) (2,1)
               wv * (2 * a1),                        # fbias1
               np.full(128, -0.5 * np.pi)]           # bnhpi
    cslots = np.zeros((128, 2, 32), np.float16)
    for i, v in enumerate(fconsts):
        cslots[:, 0, 2 * i : 2 * i + 2] = (
            np.asarray(v, np.float32).reshape(128, 1).view(np.float16)
        )

    in_maps = []
    for core in range(N_CORES):
        b, qh = divmod(core, 2)
        # keysT[p, khalf, c, 0:KH] = keys[b][khalf*512+k', c*128+p]; wk0 tail
        keysTk = keys[b].T.reshape(2, 128, 2, KH).transpose(1, 2, 0, 3)
        keysT = np.zeros((128, 2, 2, KH + H), np.float16)
        keysT[:, :, :, 0:KH] = keysTk
        keysT[:, 0, :, KH:] = wk0
        keysT[:, 1, :, KH:] = wk0
        qT = np.ascontiguousarray(
            queries[b, qh * QL : (qh + 1) * QL, :].T.reshape(2, 128, QL)
            .transpose(1, 0, 2)
        ).astype(np.float16)
        constA = np.concatenate([wq0, qT, cslots], axis=2)
        valT = np.ascontiguousarray(
            values[b].T.reshape(KC, 128, D).transpose(1, 0, 2)
        ).astype(np.float16)
        in_maps.append(
            {
                "constA": np.ascontiguousarray(constA),
                "constB": constB,
                "keysT": keysT,
                "valT": valT,
            }
        )
    return in_maps


def _run(queries, keys, values, W_q, W_k, w_v, trace=False):
    global _nc_cache
    if _nc_cache is None:
        _nc_cache = build()
    nc = _nc_cache
    in_maps = _make_in_maps(queries, keys, values, W_q, W_k, w_v)
    res = run_bass_kernel_spmd(nc, in_maps, core_ids=list(range(N_CORES)), trace=trace)
    out = np.empty((B, Q, D), np.float32)
    for c in range(N_CORES):
        b, qh = divmod(c, 2)
        out[b, qh * QL : (qh + 1) * QL, :] = res.results[c]["out"].astype(np.float32)
    return out, res


def kernel(queries, keys, values, W_q, W_k, w_v):
    out, _ = _run(queries, keys, values, W_q, W_k, w_v, trace=False)
    return out
